# revision 13
# baseline (speedup 1.0000x reference)
"""Trainium2 Bass kernel for nn_ADMMBlock (gnn_message_passing).

Strategy: node-parallel over 8 NeuronCores. Nodes are permuted (in-degree
balanced across cores, sorted desc within a core) and padded to 2560/core.
All neighbor gathers become indirect DMAs against full node tables that live
in core-local DRAM; tables are refreshed with AllGather collectives whenever
the sharded state vector changes. The reverse operator Ldr^T is computed as a
gather over a host-built reverse-CSR (padded per 128-node chunk), so no
scatter is needed on device.

Self-contained: imports only numpy + the concourse toolchain from
/opt/trn_rl_repo (part of the environment, not the problem dir).
"""

import os
import sys

import numpy as np

if "/opt/trn_rl_repo" not in sys.path:
    sys.path.insert(0, "/opt/trn_rl_repo")

# ---------------- problem constants (hardcoded) ----------------
N = 20000
K = 10
T = 12
H = 4
C = 4
HC = H * C  # 16
TW = T * HC  # 192
NCORE = 8
S = 2560  # padded nodes per core
NP_ = NCORE * S  # 20480
CH = 128
NCHUNK = S // CH  # 20
ADMM = 3
CG = 3
RD = 12  # reverse-CSR slots handled per gather round


# ---------------- host-side prep ----------------
def build_prep(y, u_ew, d_ew, nearest_nodes):
    nn = np.asarray(nearest_nodes).astype(np.int64)
    y = np.asarray(y, dtype=np.float32)
    u_ew = np.asarray(u_ew, dtype=np.float32)
    d_ew = np.asarray(d_ew, dtype=np.float32)

    deg = np.bincount(nn.ravel(), minlength=N)
    order = np.argsort(-deg, kind="stable")
    core_of_rank = np.tile(
        np.concatenate([np.arange(NCORE), np.arange(NCORE)[::-1]]),
        N // (2 * NCORE) + 1,
    )[:N]
    orig = np.full(NP_, -1, dtype=np.int64)
    for c in range(NCORE):
        cn = order[core_of_rank == c]
        orig[c * S : c * S + len(cn)] = cn
    old2new = np.full(N, -1, dtype=np.int64)
    valid = orig >= 0
    old2new[orig[valid]] = np.nonzero(valid)[0]

    real = valid
    o = np.where(real, orig, 0)

    x0 = np.zeros((NP_, T, H, C), dtype=np.float32)
    xo = np.transpose(y[0][:, o[real], :], (1, 0, 2))  # (nreal, T, C)
    x0[real] = np.broadcast_to(xo[:, :, None, :], (xo.shape[0], T, H, C))
    x0_flat = np.ascontiguousarray(x0.reshape(NP_, TW))

    fwd_idx = np.zeros((NP_, K), dtype=np.int32)
    fwd_idx[real] = old2new[nn[o[real]]].astype(np.int32)

    UW = np.zeros((NP_, K - 1, T, H), dtype=np.float32)
    UW[real] = np.transpose(u_ew[0][:, o[real], :, :], (1, 2, 0, 3))
    UW = np.ascontiguousarray(UW.reshape(NP_, (K - 1) * T * H))
    DW = np.zeros((NP_, K, T - 1, H), dtype=np.float32)
    DW[real] = np.transpose(d_ew[0][:, o[real], :, :], (1, 2, 0, 3))
    DW = np.ascontiguousarray(DW.reshape(NP_, K * (T - 1) * H))

    # reverse CSR (new ids), padded per 128-chunk
    src_old = np.repeat(np.arange(N), K)
    k_of_e = np.tile(np.arange(K), N)
    dst_new = old2new[nn.ravel()]
    src_new = old2new[src_old]
    eorder = np.argsort(dst_new, kind="stable")
    dst_s = dst_new[eorder]
    src_s = src_new[eorder]
    me_s = src_old[eorder]
    ke_s = k_of_e[eorder]
    rdeg = np.bincount(dst_new, minlength=NP_)
    starts = np.concatenate([[0], np.cumsum(rdeg)])
    D = rdeg.reshape(NCORE, NCHUNK, CH).max(axis=(0, 2)).astype(np.int64)
    SD = int(D.sum())
    doff = np.concatenate([[0], np.cumsum(D)])
    rev_idx = np.zeros((NP_, SD), dtype=np.int32)
    RW = np.zeros((NP_, SD, T - 1, H), dtype=np.float32)
    pos_in_dst = np.arange(len(dst_s)) - starts[dst_s]
    chunk_of_dst = (dst_s % S) // CH
    col = doff[chunk_of_dst] + pos_in_dst
    rev_idx[dst_s, col] = src_s.astype(np.int32)
    RW[dst_s, col] = np.transpose(d_ew[0][:, me_s, ke_s, :], (1, 0, 2))
    RW = np.ascontiguousarray(RW.reshape(NP_, SD * (T - 1) * H))

    return dict(
        orig=orig, real=real, x0=x0_flat, fwd_idx=fwd_idx, UW=UW, DW=DW,
        D=D, SD=SD, rev_idx=rev_idx, RW=RW,
    )


def host_init(prep, sc):
    """Iteration-0 init on host: phi0 = Ldr(x0), r0 = rhs0 - lhs(x0)."""
    x0 = prep["x0"]
    fwd_idx = prep["fwd_idx"]
    UWv = prep["UW"].reshape(NP_, K - 1, T, H)
    DWv = prep["DW"].reshape(NP_, K, T - 1, H)
    RWv = prep["RW"].reshape(NP_, prep["SD"], T - 1, H)
    rev_idx = prep["rev_idx"]
    mask = sc["mask"]
    mu_u0, rho0 = sc["mu_u"][0], sc["rho"][0]
    cc0 = sc["mu_d2"][0] + rho0 / 2.0

    g = x0[fwd_idx].reshape(NP_, K, T, H, C)
    lu = x0.reshape(NP_, T, H, C) - np.einsum(
        "nkth,nkthc->nthc", UWv, g[:, 1:], optimize=True
    )
    dacc = np.einsum("nkth,nkthc->nthc", DWv, g[:, :, : T - 1], optimize=True)
    phi0 = np.zeros((NP_, T, H, C), np.float32)
    phi0[:, 1:] = x0.reshape(NP_, T, H, C)[:, 1:] - dacc
    del g

    def ldr_t(w):
        out = np.empty((NP_, T, H, C), np.float32)
        wv = w.reshape(NP_, T, H, C)
        for c in range(NCORE):
            sl = slice(c * S, (c + 1) * S)
            gb = w.reshape(NP_, TW)[rev_idx[sl]].reshape(S, -1, T, H, C)
            scat = np.einsum("ndth,ndthc->nthc", RWv[sl], gb[:, :, 1:], optimize=True)
            out[sl, 0] = -scat[:, 0]
            out[sl, 1 : T - 1] = wv[sl, 1 : T - 1] - scat[:, 1:]
            out[sl, T - 1] = wv[sl, T - 1]
        return out

    Hty = x0.reshape(NP_, T, H, C).copy()
    Hty[:, mask:] = 0.0
    w0 = (rho0 * phi0 + 0.1).astype(np.float32)
    rhs = ldr_t(w0.reshape(NP_, TW)) / 2.0 + Hty
    cldr_x = ldr_t(phi0.reshape(NP_, TW))
    hthx = x0.reshape(NP_, T, H, C).copy()
    hthx[:, mask:] = 0.0
    r0 = rhs - (hthx + mu_u0 * lu + cc0 * cldr_x)
    return (
        np.ascontiguousarray(r0.reshape(NP_, TW).astype(np.float32)),
        np.ascontiguousarray(phi0.reshape(NP_, TW).astype(np.float32)),
    )


# ---------------- device kernel builder ----------------
def build_bass(sc, D, SD):
    """sc: dict with python-number scalars: mu_u[i], mu_d1[i], mu_d2[i],
    rho[i], alpha[i][j][h], beta[i][j][h], comb[h], mask."""
    import concourse.bass as bass
    import concourse.bacc as bacc
    from concourse import mybir, tile

    f32 = mybir.dt.float32
    bf16 = mybir.dt.bfloat16
    i32 = mybir.dt.int32
    i16 = mybir.dt.int16
    AL = mybir.AluOpType
    ACT = mybir.ActivationFunctionType

    mask = int(sc["mask"])
    MW = mask * HC  # masked column width (t < mask)

    nc = bacc.Bacc(None, num_devices=NCORE)
    rg = [list(range(NCORE))]

    # --- kernel I/O ---
    x0s = nc.declare_dram_parameter("x0s", [S, TW], f32, isOutput=False)
    r0s = nc.declare_dram_parameter("r0s", [S, TW], f32, isOutput=False)
    phi0s = nc.declare_dram_parameter("phi0s", [S, TW], f32, isOutput=False)
    fwdw = nc.declare_dram_parameter("fwdw", [CH, NCHUNK * CH * K // 16], i16, isOutput=False)
    selfw = nc.declare_dram_parameter("selfw", [CH, NCHUNK * CH // 16], i16, isOutput=False)
    uw = nc.declare_dram_parameter("uw", [S, (K - 1) * T * H], f32, isOutput=False)
    dw = nc.declare_dram_parameter("dw", [S, K * (T - 1) * H], f32, isOutput=False)
    rw = nc.declare_dram_parameter("rw", [S, SD * (T - 1) * H], f32, isOutput=False)
    revw = nc.declare_dram_parameter("revw", [CH, SD * CH // 16], i16, isOutput=False)
    outp = nc.declare_dram_parameter("out", [S, T * C], f32, isOutput=True)

    # --- internal DRAM: AG bounce inputs + shared tables ---
    n_ag = (ADMM - 1) + ADMM * 4 + (ADMM - 1) * 2  # w(i>0) + CG(p,l)x2 + boundary(x,ldrx)
    agin = [nc.dram_tensor(f"agin{z}", [S, TW], f32) for z in range(n_ag)]
    tabs = [
        nc.dram_tensor(f"tab{z}", [NP_, TW], f32, addr_space="Shared")
        for z in range(n_ag)
    ]
    _ag_z = [0]

    doff = np.concatenate([[0], np.cumsum(D)]).astype(int)

    with tile.TileContext(nc) as tc:
        with (
            tc.tile_pool(name="state", bufs=1) as statep,
            tc.tile_pool(name="gfp", bufs=2) as gfp,
            tc.tile_pool(name="grp", bufs=2) as grp,
            tc.tile_pool(name="prodp", bufs=2) as prodp,
            tc.tile_pool(name="wtp", bufs=2) as wtp,
            tc.tile_pool(name="accp", bufs=2) as accp,
            tc.tile_pool(name="scatp", bufs=2) as scatp,
            tc.tile_pool(name="smallp", bufs=3) as smallp,
        ):
            # --- persistent state tiles (one slot per unique tag) ---
            xs = statep.tile([CH, NCHUNK, T, H, C], f32, name="xs", tag="xs")
            rs = statep.tile([CH, NCHUNK, T, H, C], f32, name="rs", tag="rs")
            ps = statep.tile([CH, NCHUNK, T, H, C], f32, name="ps", tag="ps")
            lus = statep.tile([CH, NCHUNK, T, H, C], f32, name="lus", tag="lus")
            hty = statep.tile([CH, NCHUNK, mask, H, C], f32, name="hty", tag="hty")
            gam = statep.tile([CH, NCHUNK, T, H, C], bf16, name="gam", tag="gam")
            phi = statep.tile([CH, NCHUNK, T, H, C], bf16, name="phi", tag="phi")
            fidxr = statep.tile([CH, NCHUNK * CH * K // 16], i16, name="fidxr", tag="fidxr")
            ridxr = statep.tile([CH, SD * CH // 16], i16, name="ridxr", tag="ridxr")
            sidxr = statep.tile([CH, NCHUNK * CH // 16], i16, name="sidxr", tag="sidxr")

            # ---- load initial state ----
            nc.sync.dma_start(
                out=xs[:].rearrange("p ch t h c -> p ch (t h c)"),
                in_=x0s[:, :].rearrange("(ch p) w -> p ch w", p=CH),
            )
            nc.sync.dma_start(out=fidxr[:, :], in_=fwdw[:, :])
            nc.sync.dma_start(out=ridxr[:, :], in_=revw[:, :])
            nc.sync.dma_start(out=sidxr[:, :], in_=selfw[:, :])
            nc.vector.tensor_copy(out=hty[:], in_=xs[:, :, 0:mask])
            nc.gpsimd.memset(gam[:], 0.1)
            nc.sync.dma_start(
                out=rs[:].rearrange("p ch t h c -> p ch (t h c)"),
                in_=r0s[:, :].rearrange("(ch p) w -> p ch w", p=CH),
            )
            nc.gpsimd.dma_start(
                out=phi[:].rearrange("p ch t h c -> p ch (t h c)"),
                in_=phi0s[:, :].rearrange("(ch p) w -> p ch w", p=CH),
            )

            def new_ag():
                z = _ag_z[0]
                _ag_z[0] += 1
                return agin[z], tabs[z]

            def run_ag(src_tile_ap, dst=None):
                """DMA a full-shard SBUF AP -> agin, AllGather -> tab."""
                ag_in, tab = new_ag() if dst is None else dst
                nc.sync.dma_start(
                    out=ag_in[:, :].rearrange("(ch p) w -> p ch w", p=CH),
                    in_=src_tile_ap,
                )
                nc.gpsimd.collective_compute(
                    "AllGather", AL.bypass, replica_groups=rg,
                    ins=[ag_in[:, :].opt()], outs=[tab[:, :].opt()],
                )
                return tab

            # ---------------- forward pass ----------------
            def fwd_pass(tab, mu_u_val, vec, mode, bctx=None):
                """Gathers tab rows for all fwd edges; computes
                lus := mu_u*Lu(vec) and ldr := Ldr(vec) per chunk.
                mode: 'init' (store phi), 'cg' (nothing extra),
                'boundary' (phi/gamma update via bctx=(rho, thr)).
                Returns the table of Ldr(vec) (AllGathered from chunks)."""
                ag_in, ltab = new_ag()
                FC = CH * K // 16  # idx cols per chunk (80)
                for b in range(NCHUNK):
                    gf = gfp.tile([CH, K, T, H, C], f32, tag="gf")
                    nc.gpsimd.dma_gather(
                        out_ap=gf[:].rearrange("p k t h c -> p k (t h c)"),
                        in_ap=tab[:, :],
                        idxs_ap=fidxr[:, b * FC : (b + 1) * FC],
                        num_idxs=CH * K,
                        num_idxs_reg=CH * K,
                        elem_size=TW,
                        single_packet=False,
                    )
                    uwt = wtp.tile([CH, K - 1, T, H], f32, tag="uwt")
                    nc.sync.dma_start(
                        out=uwt[:].rearrange("p k t h -> p (k t h)"),
                        in_=uw[b * CH : (b + 1) * CH, :],
                    )
                    dwt = wtp.tile([CH, K, T - 1, H], f32, tag="dwt")
                    nc.sync.dma_start(
                        out=dwt[:].rearrange("p k t h -> p (k t h)"),
                        in_=dw[b * CH : (b + 1) * CH, :],
                    )
                    if mu_u_val != 1.0 and not sc.get("mu_u_baked"):
                        nc.vector.tensor_scalar_mul(uwt[:], uwt[:], float(mu_u_val))
                    prod = prodp.tile([CH, C, RD, 48], f32, tag="prod")
                    # Lu: k=1..9, all t
                    for c in range(C):
                        nc.vector.tensor_tensor(
                            out=prod[:, c, 0 : K - 1, 0 : T * H].rearrange(
                                "p k (t h) -> p k t h", t=T
                            ),
                            in0=gf[:, 1:, :, :, c],
                            in1=uwt[:],
                            op=AL.mult,
                        )
                    luacc = accp.tile([CH, C, T, H], f32, tag="luacc")
                    nc.vector.tensor_reduce(
                        out=luacc[:],
                        in_=prod[:, :, 0 : K - 1, 0 : T * H].rearrange(
                            "p c k j -> p c j k"
                        ),
                        axis=mybir.AxisListType.X,
                        op=AL.add,
                    )
                    # lus = mu_u * vec - luacc(scaled)
                    nc.vector.scalar_tensor_tensor(
                        out=lus[:, b],
                        in0=vec[:, b],
                        scalar=float(mu_u_val),
                        in1=luacc[:].rearrange("p c t h -> p t h c"),
                        op0=AL.mult,
                        op1=AL.subtract,
                    )
                    # Ldr: k=0..9, t=0..10 -> out t=1..11
                    for c in range(C):
                        nc.vector.tensor_tensor(
                            out=prod[:, c, 0:K, 0 : (T - 1) * H].rearrange(
                                "p k (t h) -> p k t h", t=T - 1
                            ),
                            in0=gf[:, :, 0 : T - 1, :, c],
                            in1=dwt[:],
                            op=AL.mult,
                        )
                    dracc = accp.tile([CH, C, T - 1, H], f32, tag="dracc")
                    nc.vector.tensor_reduce(
                        out=dracc[:],
                        in_=prod[:, :, 0:K, 0 : (T - 1) * H].rearrange(
                            "p c k j -> p c j k"
                        ),
                        axis=mybir.AxisListType.X,
                        op=AL.add,
                    )
                    ldc = smallp.tile([CH, T, H, C], f32, tag="ldc")
                    nc.vector.memset(ldc[:, 0], 0.0)
                    nc.vector.tensor_tensor(
                        out=ldc[:, 1:],
                        in0=vec[:, b, 1:],
                        in1=dracc[:].rearrange("p c t h -> p t h c"),
                        op=AL.subtract,
                    )
                    nc.sync.dma_start(
                        out=ag_in[b * CH : (b + 1) * CH, :],
                        in_=ldc[:].rearrange("p t h c -> p (t h c)"),
                    )
                    if mode == "init":
                        nc.vector.tensor_copy(out=phi[:, b], in_=ldc[:])
                    elif mode == "boundary":
                        rho_v, thr = bctx
                        st = smallp.tile([CH, T, H, C], f32, tag="st")
                        nc.vector.scalar_tensor_tensor(
                            out=st[:], in0=gam[:, b], scalar=float(-1.0 / rho_v),
                            in1=ldc[:], op0=AL.mult, op1=AL.add,
                        )
                        t1 = smallp.tile([CH, T, H, C], f32, tag="t1")
                        nc.vector.tensor_scalar(
                            out=t1[:], in0=st[:], scalar1=float(-thr),
                            scalar2=0.0, op0=AL.add, op1=AL.max,
                        )
                        t2 = smallp.tile([CH, T, H, C], f32, tag="t2")
                        nc.vector.tensor_scalar(
                            out=t2[:], in0=st[:], scalar1=float(thr),
                            scalar2=0.0, op0=AL.add, op1=AL.min,
                        )
                        nc.vector.tensor_tensor(
                            out=phi[:, b], in0=t1[:], in1=t2[:], op=AL.add
                        )
                        # gamma += rho*(phi - ldr)
                        g1 = smallp.tile([CH, T, H, C], f32, tag="g1")
                        nc.vector.tensor_tensor(
                            out=g1[:], in0=phi[:, b], in1=ldc[:], op=AL.subtract
                        )
                        nc.vector.scalar_tensor_tensor(
                            out=gam[:, b], in0=g1[:], scalar=float(rho_v),
                            in1=gam[:, b], op0=AL.mult, op1=AL.add,
                        )
                nc.gpsimd.collective_compute(
                    "AllGather", AL.bypass, replica_groups=rg,
                    ins=[ag_in[:, :].opt()], outs=[ltab[:, :].opt()],
                )
                return ltab

            # ---------------- reverse pass ----------------
            def rev_pass(tab, combine):
                """combine(b, scat, wch_getter) writes results for chunk b.
                scat: [CH, C, T, H] tile; scat[:, :, T-1] is zero;
                scat[:, :, t] = sum_d RW*g[t+1] for t=0..10."""
                for b in range(NCHUNK):
                    db = int(D[b])
                    scat = scatp.tile([CH, C, T, H], f32, tag="scat")
                    nc.vector.memset(scat[:, :, T - 1], 0.0)
                    nrounds = max(1, (db + RD - 1) // RD)
                    for r in range(nrounds):
                        lo = r * RD
                        cnt = min(RD, db - lo)
                        if cnt <= 0:
                            nc.vector.memset(scat[:, :, 0 : T - 1], 0.0)
                            break
                        rwt = wtp.tile([CH, RD, T - 1, H], f32, tag="rwt")
                        nc.sync.dma_start(
                            out=rwt[:, 0:cnt].rearrange("p d t h -> p (d t h)"),
                            in_=rw[
                                b * CH : (b + 1) * CH,
                                (doff[b] + lo) * (T - 1) * H : (doff[b] + lo + cnt) * (T - 1) * H,
                            ],
                        )
                        gr = grp.tile([CH, RD, T, H, C], f32, tag="gr")
                        nc.gpsimd.dma_gather(
                            out_ap=gr[:, 0:cnt].rearrange("p d t h c -> p d (t h c)"),
                            in_ap=tab[:, :],
                            idxs_ap=ridxr[:, (doff[b] + lo) * 8 : (doff[b] + lo + cnt) * 8],
                            num_idxs=CH * cnt,
                            num_idxs_reg=CH * cnt,
                            elem_size=TW,
                            single_packet=False,
                        )
                        prod = prodp.tile([CH, C, RD, 48], f32, tag="prod")
                        for c in range(C):
                            nc.vector.tensor_tensor(
                                out=prod[:, c, 0:cnt, 0 : (T - 1) * H].rearrange(
                                    "p d (t h) -> p d t h", t=T - 1
                                ),
                                in0=gr[:, 0:cnt, 1:, :, c],
                                in1=rwt[:, 0:cnt],
                                op=AL.mult,
                            )
                        if r == 0:
                            nc.vector.tensor_reduce(
                                out=scat[:, :, 0 : T - 1],
                                in_=prod[:, :, 0:cnt, 0 : (T - 1) * H].rearrange(
                                    "p c d j -> p c j d"
                                ),
                                axis=mybir.AxisListType.X,
                                op=AL.add,
                            )
                        else:
                            sc2 = accp.tile([CH, C, T - 1, H], f32, tag="sc2")
                            nc.vector.tensor_reduce(
                                out=sc2[:],
                                in_=prod[:, :, 0:cnt, 0 : (T - 1) * H].rearrange(
                                    "p c d j -> p c j d"
                                ),
                                axis=mybir.AxisListType.X,
                                op=AL.add,
                            )
                            nc.vector.tensor_tensor(
                                out=scat[:, :, 0 : T - 1],
                                in0=scat[:, :, 0 : T - 1],
                                in1=sc2[:],
                                op=AL.add,
                            )
                    combine(b, scat)

            def self_gather(tab, b):
                wch = smallp.tile([CH, 1, T, H, C], f32, tag="wchg")
                nc.gpsimd.dma_gather(
                    out_ap=wch[:].rearrange("p o t h c -> p o (t h c)"),
                    in_ap=tab[:, :],
                    idxs_ap=sidxr[:, b * 8 : (b + 1) * 8],
                    num_idxs=CH,
                    num_idxs_reg=CH,
                    elem_size=TW,
                    single_packet=False,
                )
                return wch[:, 0]

            def cldr_from(scat, wch, qt):
                """qt = Ldr_T(w) in one op: requires wch[t=0] == 0 (true for
                Ldr outputs; rhs zeroes it explicitly). scat[T-1] is zero so
                t=11 passes w through; t=0 gives 0 - scat[0]."""
                nc.vector.tensor_tensor(
                    out=qt[:, :],
                    in0=wch[:, :],
                    in1=scat[:, :, :].rearrange("p c t h -> p t h c"),
                    op=AL.subtract,
                )

            # ================= program =================
            ldr_tab = None  # iteration-0 rhs/r-init is done on host

            flat = lambda ap: ap.rearrange("p t h c -> p (t h c)")

            for i in range(ADMM):
                rho_v = sc["rho"][i]
                cc = sc["mu_d2"][i] + rho_v / 2.0
                if i == 0:
                    nc.vector.tensor_copy(
                        out=ps[:].rearrange("p ch t h c -> p (ch t h c)"),
                        in_=rs[:].rearrange("p ch t h c -> p (ch t h c)"),
                    )
                # ---- rhs: r = LdrT(rho*phi + gamma)/2 + Hty ----
                if i == 0:
                    pass
                else:
                    ag_in, wtab = new_ag()
                    for b in range(NCHUNK):
                        wch = smallp.tile([CH, T, H, C], f32, tag="wch")
                        nc.vector.scalar_tensor_tensor(
                            out=wch[:], in0=phi[:, b], scalar=float(rho_v),
                            in1=gam[:, b], op0=AL.mult, op1=AL.add,
                        )
                        nc.sync.dma_start(
                            out=ag_in[b * CH : (b + 1) * CH, :], in_=flat(wch[:])
                        )
                    nc.gpsimd.collective_compute(
                        "AllGather", AL.bypass, replica_groups=rg,
                        ins=[ag_in[:, :].opt()], outs=[wtab[:, :].opt()],
                    )

                def rhs_combine(b, scat, _i=i, _rho=rho_v):
                    wch = smallp.tile([CH, T, H, C], f32, tag="wch")
                    nc.vector.memset(wch[:, 0], 0.0)
                    nc.vector.scalar_tensor_tensor(
                        out=wch[:, 1:], in0=phi[:, b, 1:], scalar=float(_rho),
                        in1=gam[:, b, 1:], op0=AL.mult, op1=AL.add,
                    )
                    cldr_from(scat, wch[:], rs[:, b])
                    nc.vector.scalar_tensor_tensor(
                        out=rs[:, b, 0:mask].rearrange("p t h c -> p (t h c)"),
                        in0=rs[:, b, 0:mask].rearrange("p t h c -> p (t h c)"),
                        scalar=0.5,
                        in1=hty[:, b].rearrange("p t h c -> p (t h c)"),
                        op0=AL.mult,
                        op1=AL.add,
                    )
                    nc.vector.tensor_scalar_mul(
                        rs[:, b, mask:].rearrange("p t h c -> p (t h c)"),
                        rs[:, b, mask:].rearrange("p t h c -> p (t h c)"),
                        0.5,
                    )

                if i > 0:
                    rev_pass(wtab, rhs_combine)

                # ---- r -= lhs(x) ----
                def rinit_combine(b, scat, _cc=cc, _lt=ldr_tab):
                    wch = self_gather(_lt, b)
                    qt = smallp.tile([CH, T, H, C], f32, tag="qt")
                    cldr_from(scat, wch, qt)
                    nc.vector.scalar_tensor_tensor(
                        out=flat(rs[:, b]), in0=flat(qt[:]), scalar=float(-_cc),
                        in1=flat(rs[:, b]), op0=AL.mult, op1=AL.add,
                    )
                    nc.vector.tensor_tensor(
                        out=flat(rs[:, b]), in0=flat(rs[:, b]),
                        in1=flat(lus[:, b]), op=AL.subtract,
                    )
                    nc.vector.tensor_tensor(
                        out=rs[:, b, 0:mask],
                        in0=rs[:, b, 0:mask],
                        in1=xs[:, b, 0:mask],
                        op=AL.subtract,
                    )

                if i > 0:
                    rev_pass(ldr_tab, rinit_combine)
                    nc.vector.tensor_copy(
                        out=ps[:].rearrange("p ch t h c -> p (ch t h c)"),
                        in_=rs[:].rearrange("p ch t h c -> p (ch t h c)"),
                    )

                # ---- CG ----
                for j in range(CG):
                    al = sc["alpha"][i][j]  # list of H floats
                    be = sc["beta"][i][j]
                    al_eq = all(a == al[0] for a in al)
                    be_eq = all(b2 == be[0] for b2 in be)
                    if j == CG - 1:
                        # only x += a*p
                        if al_eq:
                            nc.vector.scalar_tensor_tensor(
                                out=xs[:].rearrange("p ch t h c -> p (ch t h c)"),
                                in0=ps[:].rearrange("p ch t h c -> p (ch t h c)"),
                                scalar=float(al[0]),
                                in1=xs[:].rearrange("p ch t h c -> p (ch t h c)"),
                                op0=AL.mult, op1=AL.add,
                            )
                        else:
                            for h in range(H):
                                nc.vector.scalar_tensor_tensor(
                                    out=xs[:, :, :, h],
                                    in0=ps[:, :, :, h],
                                    scalar=float(al[h]),
                                    in1=xs[:, :, :, h],
                                    op0=AL.mult, op1=AL.add,
                                )
                        break
                    ptab = run_ag(ps[:].rearrange("p ch t h c -> p ch (t h c)"))
                    ltab_p = fwd_pass(ptab, sc["mu_u"][i], ps, "cg")

                    def cg_combine(b, scat, _cc=cc, _al=al, _be=be, _j=j,
                                   _lt=ltab_p, _aleq=al_eq, _beeq=be_eq):
                        wch = self_gather(_lt, b)
                        qt = smallp.tile([CH, T, H, C], f32, tag="qt")
                        cldr_from(scat, wch, qt)
                        # q = cc*cldr + lus
                        nc.vector.scalar_tensor_tensor(
                            out=flat(qt[:]), in0=flat(qt[:]), scalar=float(_cc),
                            in1=flat(lus[:, b]), op0=AL.mult, op1=AL.add,
                        )
                        # r -= a*q ; r[t<mask] -= a*p
                        if _aleq:
                            nc.vector.scalar_tensor_tensor(
                                out=flat(rs[:, b]), in0=flat(qt[:]),
                                scalar=float(-_al[0]), in1=flat(rs[:, b]),
                                op0=AL.mult, op1=AL.add,
                            )
                            nc.vector.scalar_tensor_tensor(
                                out=rs[:, b, 0:mask].rearrange("p t h c -> p (t h c)"),
                                in0=ps[:, b, 0:mask].rearrange("p t h c -> p (t h c)"),
                                scalar=float(-_al[0]),
                                in1=rs[:, b, 0:mask].rearrange("p t h c -> p (t h c)"),
                                op0=AL.mult, op1=AL.add,
                            )
                            nc.vector.scalar_tensor_tensor(
                                out=flat(xs[:, b]), in0=flat(ps[:, b]),
                                scalar=float(_al[0]), in1=flat(xs[:, b]),
                                op0=AL.mult, op1=AL.add,
                            )
                        else:
                            for h in range(H):
                                fl = lambda ap: ap
                                nc.vector.scalar_tensor_tensor(
                                    out=fl(rs[:, b, :, h]), in0=fl(qt[:, :, h]),
                                    scalar=float(-_al[h]), in1=fl(rs[:, b, :, h]),
                                    op0=AL.mult, op1=AL.add,
                                )
                                nc.vector.scalar_tensor_tensor(
                                    out=fl(rs[:, b, 0:mask, h]),
                                    in0=fl(ps[:, b, 0:mask, h]),
                                    scalar=float(-_al[h]),
                                    in1=fl(rs[:, b, 0:mask, h]),
                                    op0=AL.mult, op1=AL.add,
                                )
                                nc.vector.scalar_tensor_tensor(
                                    out=fl(xs[:, b, :, h]), in0=fl(ps[:, b, :, h]),
                                    scalar=float(_al[h]), in1=fl(xs[:, b, :, h]),
                                    op0=AL.mult, op1=AL.add,
                                )
                        # p = r + b*p
                        if _beeq:
                            nc.vector.scalar_tensor_tensor(
                                out=flat(ps[:, b]), in0=flat(ps[:, b]),
                                scalar=float(_be[0]), in1=flat(rs[:, b]),
                                op0=AL.mult, op1=AL.add,
                            )
                        else:
                            for h in range(H):
                                fl = lambda ap: ap
                                nc.vector.scalar_tensor_tensor(
                                    out=fl(ps[:, b, :, h]), in0=fl(ps[:, b, :, h]),
                                    scalar=float(_be[h]), in1=fl(rs[:, b, :, h]),
                                    op0=AL.mult, op1=AL.add,
                                )

                    rev_pass(ltab_p, cg_combine)

                # ---- boundary ----
                if i < ADMM - 1:
                    xtab = run_ag(xs[:].rearrange("p ch t h c -> p ch (t h c)"))
                    thr = sc["mu_d1"][i] / sc["rho"][i]
                    ldr_tab = fwd_pass(
                        xtab, sc["mu_u"][i + 1], xs, "boundary",
                        bctx=(sc["rho"][i], thr),
                    )

            # ---- output: out[n, t*C+c] = sum_h comb[h]*x ----
            for b in range(NCHUNK):
                oc = smallp.tile([CH, T, C], f32, tag="oc")
                nc.vector.tensor_scalar(
                    out=oc[:],
                    in0=xs[:, b, :, 0],
                    scalar1=float(sc["comb"][0]),
                    scalar2=None,
                    op0=AL.mult,
                )
                for h in range(1, H):
                    nc.vector.scalar_tensor_tensor(
                        out=oc[:],
                        in0=xs[:, b, :, h],
                        scalar=float(sc["comb"][h]),
                        in1=oc[:],
                        op0=AL.mult, op1=AL.add,
                    )
                nc.sync.dma_start(
                    out=outp[b * CH : (b + 1) * CH, :],
                    in_=oc[:].rearrange("p t c -> p (t c)"),
                )

    assert _ag_z[0] == n_ag, (_ag_z[0], n_ag)
    nc.compile()
    return nc


# ---------------- full kernel entry ----------------
def _wrap16(vals):
    """Wrap a flat idx list: pos j -> (partition j%16, col j//16); tiled to
    128 partitions (each GpSimd core reads its own 16-partition copy)."""
    v = np.ascontiguousarray(vals.astype(np.int16))
    w = v.reshape(-1, 16).T
    return np.tile(w, (8, 1))


def _make_in_maps(inputs, prep, r0, phi0):
    x0 = prep["x0"]
    D = prep["D"]
    doff = np.concatenate([[0], np.cumsum(D)]).astype(int)
    in_maps = []
    for c in range(NCORE):
        sl = slice(c * S, (c + 1) * S)
        fwd = prep["fwd_idx"][sl]  # [S, K]
        # fwd wrapped: per chunk, position i = k*128+p
        fw = np.concatenate(
            [fwd[b * CH : (b + 1) * CH].T.reshape(-1) for b in range(NCHUNK)]
        )
        # rev wrapped: per chunk b / round r, position i = d*128+p
        rev = prep["rev_idx"][sl]  # [S, SD]
        rparts = []
        for b in range(NCHUNK):
            sub = rev[b * CH : (b + 1) * CH, doff[b] : doff[b + 1]]  # [128, D[b]]
            rparts.append(sub.T.reshape(-1))
        rv = np.concatenate(rparts) if rparts else np.zeros(0, np.int64)
        sv = c * S + np.arange(S)
        in_maps.append(
            {
                "x0s": x0[sl],
                "r0s": r0[sl],
                "phi0s": phi0[sl],
                "fwdw": _wrap16(fw),
                "selfw": _wrap16(sv),
                "revw": _wrap16(rv),
                "uw": prep["UW"][sl],
                "dw": prep["DW"][sl],
                "rw": prep["RW"][sl],
            }
        )
    return in_maps


def _scalars(inputs):
    return dict(
        mu_u=[float(v) for v in np.asarray(inputs["mu_u"])],
        mu_d1=[float(v) for v in np.asarray(inputs["mu_d1"])],
        mu_d2=[float(v) for v in np.asarray(inputs["mu_d2"])],
        rho=[float(v) for v in np.asarray(inputs["rho"])],
        alpha=[
            [[float(x) for x in np.asarray(inputs["alpha_x"])[i, j, :, 0]]
             for j in range(CG)]
            for i in range(ADMM)
        ],
        beta=[
            [[float(x) for x in np.asarray(inputs["beta_x"])[i, j, :, 0]]
             for j in range(CG)]
            for i in range(ADMM)
        ],
        comb=[float(v) for v in np.asarray(inputs["comb_weights"])],
        mask=int(inputs["mask"]),
    )


def _assemble_out(results, prep):
    out_new = np.concatenate(
        [np.asarray(results[c]["out"]) for c in range(NCORE)], axis=0
    )  # (NP_, T*C)
    out = np.zeros((1, T, N, C), np.float32)
    real = prep["real"]
    out[0, :, prep["orig"][real], :] = out_new[real].reshape(-1, T, C)
    return out


def _maybe_bake_mu_u(prep, sc):
    """If mu_u is constant across iterations, pre-scale UW on the host and
    drop the per-chunk device scale op."""
    if all(v == sc["mu_u"][0] for v in sc["mu_u"]) and sc["mu_u"][0] != 1.0:
        prep["UW"] = (prep["UW"] * np.float32(sc["mu_u"][0])).astype(np.float32)
        sc["mu_u_baked"] = True


def kernel(**inputs) -> np.ndarray:
    from concourse.bass_utils import run_bass_kernel_spmd

    prep = build_prep(
        inputs["y"], inputs["u_ew"], inputs["d_ew"], inputs["nearest_nodes"]
    )
    sc = _scalars(inputs)
    r0, phi0 = host_init(prep, sc)
    _maybe_bake_mu_u(prep, sc)
    nc = build_bass(sc, prep["D"], prep["SD"])
    in_maps = _make_in_maps(inputs, prep, r0, phi0)
    res = run_bass_kernel_spmd(nc, in_maps, core_ids=list(range(NCORE)))
    return _assemble_out(res.results, prep)


# revision 15
# speedup vs baseline: 1.3656x; 1.3656x over previous
"""Trainium2 Bass kernel for nn_ADMMBlock (gnn_message_passing).

Strategy: node-parallel over 8 NeuronCores. Nodes are permuted (in-degree
balanced across cores, sorted desc within a core) and padded to 2560/core.
All neighbor gathers become indirect DMAs against full node tables that live
in core-local DRAM; tables are refreshed with AllGather collectives whenever
the sharded state vector changes. The reverse operator Ldr^T is computed as a
gather over a host-built reverse-CSR (padded per 128-node chunk), so no
scatter is needed on device.

Self-contained: imports only numpy + the concourse toolchain from
/opt/trn_rl_repo (part of the environment, not the problem dir).
"""

import os
import sys

import numpy as np

if "/opt/trn_rl_repo" not in sys.path:
    sys.path.insert(0, "/opt/trn_rl_repo")

# ---------------- problem constants (hardcoded) ----------------
N = 20000
K = 10
T = 12
H = 4
C = 4
HC = H * C  # 16
TW = T * HC  # 192
NCORE = 8
S = 2560  # padded nodes per core
NP_ = NCORE * S  # 20480
CH = 128
NCHUNK = S // CH  # 20
ADMM = 3
CG = 3
RD = 12  # reverse-CSR slots handled per gather round


# ---------------- host-side prep ----------------
def build_prep(y, u_ew, d_ew, nearest_nodes):
    nn = np.asarray(nearest_nodes).astype(np.int64)
    y = np.asarray(y, dtype=np.float32)
    u_ew = np.asarray(u_ew, dtype=np.float32)
    d_ew = np.asarray(d_ew, dtype=np.float32)

    deg = np.bincount(nn.ravel(), minlength=N)
    order = np.argsort(-deg, kind="stable")
    core_of_rank = np.tile(
        np.concatenate([np.arange(NCORE), np.arange(NCORE)[::-1]]),
        N // (2 * NCORE) + 1,
    )[:N]
    orig = np.full(NP_, -1, dtype=np.int64)
    for c in range(NCORE):
        cn = order[core_of_rank == c]
        orig[c * S : c * S + len(cn)] = cn
    old2new = np.full(N, -1, dtype=np.int64)
    valid = orig >= 0
    old2new[orig[valid]] = np.nonzero(valid)[0]

    real = valid
    o = np.where(real, orig, 0)

    x0 = np.zeros((NP_, T, H, C), dtype=np.float32)
    xo = np.transpose(y[0][:, o[real], :], (1, 0, 2))  # (nreal, T, C)
    x0[real] = np.broadcast_to(xo[:, :, None, :], (xo.shape[0], T, H, C))
    x0_flat = np.ascontiguousarray(x0.reshape(NP_, TW))

    fwd_idx = np.zeros((NP_, K), dtype=np.int32)
    fwd_idx[real] = old2new[nn[o[real]]].astype(np.int32)

    UW = np.zeros((NP_, K - 1, T, H), dtype=np.float32)
    UW[real] = np.transpose(u_ew[0][:, o[real], :, :], (1, 2, 0, 3))
    UW = np.ascontiguousarray(UW.reshape(NP_, (K - 1) * T * H))
    DW = np.zeros((NP_, K, T - 1, H), dtype=np.float32)
    DW[real] = np.transpose(d_ew[0][:, o[real], :, :], (1, 2, 0, 3))
    DW = np.ascontiguousarray(DW.reshape(NP_, K * (T - 1) * H))

    # reverse CSR (new ids), padded per 128-chunk
    src_old = np.repeat(np.arange(N), K)
    k_of_e = np.tile(np.arange(K), N)
    dst_new = old2new[nn.ravel()]
    src_new = old2new[src_old]
    eorder = np.argsort(dst_new, kind="stable")
    dst_s = dst_new[eorder]
    src_s = src_new[eorder]
    me_s = src_old[eorder]
    ke_s = k_of_e[eorder]
    rdeg = np.bincount(dst_new, minlength=NP_)
    starts = np.concatenate([[0], np.cumsum(rdeg)])
    D = rdeg.reshape(NCORE, NCHUNK, CH).max(axis=(0, 2)).astype(np.int64)
    SD = int(D.sum())
    doff = np.concatenate([[0], np.cumsum(D)])
    rev_idx = np.zeros((NP_, SD), dtype=np.int32)
    RW = np.zeros((NP_, SD, T - 1, H), dtype=np.float32)
    pos_in_dst = np.arange(len(dst_s)) - starts[dst_s]
    chunk_of_dst = (dst_s % S) // CH
    col = doff[chunk_of_dst] + pos_in_dst
    rev_idx[dst_s, col] = src_s.astype(np.int32)
    RW[dst_s, col] = np.transpose(d_ew[0][:, me_s, ke_s, :], (1, 0, 2))
    RW = np.ascontiguousarray(RW.reshape(NP_, SD * (T - 1) * H))

    return dict(
        orig=orig, real=real, x0=x0_flat, fwd_idx=fwd_idx, UW=UW, DW=DW,
        D=D, SD=SD, rev_idx=rev_idx, RW=RW,
    )


def host_init(prep, sc):
    """Iteration-0 init on host: phi0 = Ldr(x0), r0 = rhs0 - lhs(x0)."""
    x0 = prep["x0"]
    fwd_idx = prep["fwd_idx"]
    UWv = prep["UW"].reshape(NP_, K - 1, T, H)
    DWv = prep["DW"].reshape(NP_, K, T - 1, H)
    RWv = prep["RW"].reshape(NP_, prep["SD"], T - 1, H)
    rev_idx = prep["rev_idx"]
    mask = sc["mask"]
    mu_u0, rho0 = sc["mu_u"][0], sc["rho"][0]
    cc0 = sc["mu_d2"][0] + rho0 / 2.0

    g = x0[fwd_idx].reshape(NP_, K, T, H, C)
    lu = x0.reshape(NP_, T, H, C) - np.einsum(
        "nkth,nkthc->nthc", UWv, g[:, 1:], optimize=True
    )
    dacc = np.einsum("nkth,nkthc->nthc", DWv, g[:, :, : T - 1], optimize=True)
    phi0 = np.zeros((NP_, T, H, C), np.float32)
    phi0[:, 1:] = x0.reshape(NP_, T, H, C)[:, 1:] - dacc
    del g

    def ldr_t(w):
        out = np.empty((NP_, T, H, C), np.float32)
        wv = w.reshape(NP_, T, H, C)
        for c in range(NCORE):
            sl = slice(c * S, (c + 1) * S)
            gb = w.reshape(NP_, TW)[rev_idx[sl]].reshape(S, -1, T, H, C)
            scat = np.einsum("ndth,ndthc->nthc", RWv[sl], gb[:, :, 1:], optimize=True)
            out[sl, 0] = -scat[:, 0]
            out[sl, 1 : T - 1] = wv[sl, 1 : T - 1] - scat[:, 1:]
            out[sl, T - 1] = wv[sl, T - 1]
        return out

    Hty = x0.reshape(NP_, T, H, C).copy()
    Hty[:, mask:] = 0.0
    w0 = (rho0 * phi0 + 0.1).astype(np.float32)
    rhs = ldr_t(w0.reshape(NP_, TW)) / 2.0 + Hty
    cldr_x = ldr_t(phi0.reshape(NP_, TW))
    hthx = x0.reshape(NP_, T, H, C).copy()
    hthx[:, mask:] = 0.0
    r0 = rhs - (hthx + mu_u0 * lu + cc0 * cldr_x)
    return (
        np.ascontiguousarray(r0.reshape(NP_, TW).astype(np.float32)),
        np.ascontiguousarray(phi0.reshape(NP_, TW).astype(np.float32)),
    )


# ---------------- device kernel builder ----------------
def build_bass(sc, D, SD):
    """sc: dict with python-number scalars: mu_u[i], mu_d1[i], mu_d2[i],
    rho[i], alpha[i][j][h], beta[i][j][h], comb[h], mask."""
    import concourse.bass as bass
    import concourse.bacc as bacc
    from concourse import mybir, tile

    f32 = mybir.dt.float32
    bf16 = mybir.dt.bfloat16
    i32 = mybir.dt.int32
    i16 = mybir.dt.int16
    AL = mybir.AluOpType
    ACT = mybir.ActivationFunctionType

    mask = int(sc["mask"])
    MW = mask * HC  # masked column width (t < mask)

    nc = bacc.Bacc(None, num_devices=NCORE)
    rg = [list(range(NCORE))]

    # --- kernel I/O ---
    x0s = nc.declare_dram_parameter("x0s", [S, TW], f32, isOutput=False)
    r0s = nc.declare_dram_parameter("r0s", [S, TW], f32, isOutput=False)
    phi0s = nc.declare_dram_parameter("phi0s", [S, TW], f32, isOutput=False)
    fwdw = nc.declare_dram_parameter("fwdw", [CH, NCHUNK * CH * K // 16], i16, isOutput=False)
    selfw = nc.declare_dram_parameter("selfw", [CH, NCHUNK * CH // 16], i16, isOutput=False)
    uw = nc.declare_dram_parameter("uw", [S, (K - 1) * T * H], f32, isOutput=False)
    dw = nc.declare_dram_parameter("dw", [S, K * (T - 1) * H], f32, isOutput=False)
    rw = nc.declare_dram_parameter("rw", [S, SD * (T - 1) * H], f32, isOutput=False)
    revw = nc.declare_dram_parameter("revw", [CH, SD * CH // 16], i16, isOutput=False)
    outp = nc.declare_dram_parameter("out", [S, T * C], f32, isOutput=True)

    # --- internal DRAM: AG bounce inputs + shared tables ---
    n_ag = (ADMM - 1) + ADMM * 4 + (ADMM - 1) * 2  # w(i>0) + CG(p,l)x2 + boundary(x,ldrx)
    agin = [nc.dram_tensor(f"agin{z}", [S, TW], f32) for z in range(n_ag)]
    tabs = [
        nc.dram_tensor(f"tab{z}", [NP_, TW], f32, addr_space="Shared")
        for z in range(n_ag)
    ]
    _ag_z = [0]

    doff = np.concatenate([[0], np.cumsum(D)]).astype(int)

    with tile.TileContext(nc) as tc:
        with (
            tc.tile_pool(name="state", bufs=1) as statep,
            tc.tile_pool(name="gfp", bufs=3) as gfp,
            tc.tile_pool(name="grp", bufs=3) as grp,
            tc.tile_pool(name="prodp", bufs=3) as prodp,
            tc.tile_pool(name="wtp", bufs=3) as wtp,
            tc.tile_pool(name="accp", bufs=2) as accp,
            tc.tile_pool(name="scatp", bufs=3) as scatp,
            tc.tile_pool(name="smallp", bufs=3) as smallp,
        ):
            # --- persistent state tiles (one slot per unique tag) ---
            xs = statep.tile([CH, NCHUNK, T, H, C], f32, name="xs", tag="xs")
            rs = statep.tile([CH, NCHUNK, T, H, C], f32, name="rs", tag="rs")
            ps = statep.tile([CH, NCHUNK, T, H, C], f32, name="ps", tag="ps")
            lus = statep.tile([CH, NCHUNK, T, H, C], f32, name="lus", tag="lus")
            hty = statep.tile([CH, NCHUNK, mask, H, C], f32, name="hty", tag="hty")
            gam = statep.tile([CH, NCHUNK, T, H, C], bf16, name="gam", tag="gam")
            phi = statep.tile([CH, NCHUNK, T, H, C], bf16, name="phi", tag="phi")
            fidxr = statep.tile([CH, NCHUNK * CH * K // 16], i16, name="fidxr", tag="fidxr")
            ridxr = statep.tile([CH, SD * CH // 16], i16, name="ridxr", tag="ridxr")
            sidxr = statep.tile([CH, NCHUNK * CH // 16], i16, name="sidxr", tag="sidxr")

            # ---- load initial state ----
            nc.sync.dma_start(
                out=xs[:].rearrange("p ch t h c -> p ch (t h c)"),
                in_=x0s[:, :].rearrange("(ch p) w -> p ch w", p=CH),
            )
            nc.sync.dma_start(out=fidxr[:, :], in_=fwdw[:, :])
            nc.sync.dma_start(out=ridxr[:, :], in_=revw[:, :])
            nc.sync.dma_start(out=sidxr[:, :], in_=selfw[:, :])
            nc.vector.tensor_copy(out=hty[:], in_=xs[:, :, 0:mask])
            nc.gpsimd.memset(gam[:], 0.1)
            nc.sync.dma_start(
                out=rs[:].rearrange("p ch t h c -> p ch (t h c)"),
                in_=r0s[:, :].rearrange("(ch p) w -> p ch w", p=CH),
            )
            nc.gpsimd.dma_start(
                out=phi[:].rearrange("p ch t h c -> p ch (t h c)"),
                in_=phi0s[:, :].rearrange("(ch p) w -> p ch w", p=CH),
            )

            def new_ag():
                z = _ag_z[0]
                _ag_z[0] += 1
                return agin[z], tabs[z]

            def run_ag(src_tile_ap, dst=None):
                """DMA a full-shard SBUF AP -> agin, AllGather -> tab."""
                ag_in, tab = new_ag() if dst is None else dst
                nc.sync.dma_start(
                    out=ag_in[:, :].rearrange("(ch p) w -> p ch w", p=CH),
                    in_=src_tile_ap,
                )
                nc.gpsimd.collective_compute(
                    "AllGather", AL.bypass, replica_groups=rg,
                    ins=[ag_in[:, :].opt()], outs=[tab[:, :].opt()],
                )
                return tab

            # ---------------- forward pass ----------------
            def fwd_pass(tab, mu_u_val, vec, mode, bctx=None):
                """Gathers tab rows for all fwd edges; computes
                lus := mu_u*Lu(vec) and ldr := Ldr(vec) per chunk.
                mode: 'init' (store phi), 'cg' (nothing extra),
                'boundary' (phi/gamma update via bctx=(rho, thr)).
                Returns the table of Ldr(vec) (AllGathered from chunks)."""
                ag_in, ltab = new_ag()
                FC = CH * K // 16  # idx cols per chunk (80)
                for b in range(NCHUNK):
                    gf = gfp.tile([CH, K, T, H, C], f32, tag="gf")
                    nc.gpsimd.dma_gather(
                        out_ap=gf[:].rearrange("p k t h c -> p k (t h c)"),
                        in_ap=tab[:, :],
                        idxs_ap=fidxr[:, b * FC : (b + 1) * FC],
                        num_idxs=CH * K,
                        num_idxs_reg=CH * K,
                        elem_size=TW,
                        single_packet=False,
                    )
                    uwt = wtp.tile([CH, K - 1, T, H], f32, tag="uwt")
                    nc.sync.dma_start(
                        out=uwt[:].rearrange("p k t h -> p (k t h)"),
                        in_=uw[b * CH : (b + 1) * CH, :],
                    )
                    dwt = wtp.tile([CH, K, T - 1, H], f32, tag="dwt")
                    nc.sync.dma_start(
                        out=dwt[:].rearrange("p k t h -> p (k t h)"),
                        in_=dw[b * CH : (b + 1) * CH, :],
                    )
                    if mu_u_val != 1.0 and not sc.get("mu_u_baked"):
                        nc.vector.tensor_scalar_mul(uwt[:], uwt[:], float(mu_u_val))
                    prod = prodp.tile([CH, C, RD, 48], f32, tag="prod")
                    # Lu: k=1..9, all t
                    for c in range(C):
                        nc.vector.tensor_tensor(
                            out=prod[:, c, 0 : K - 1, 0 : T * H].rearrange(
                                "p k (t h) -> p k t h", t=T
                            ),
                            in0=gf[:, 1:, :, :, c],
                            in1=uwt[:],
                            op=AL.mult,
                        )
                    luacc = accp.tile([CH, C, T, H], f32, tag="luacc")
                    nc.vector.tensor_reduce(
                        out=luacc[:],
                        in_=prod[:, :, 0 : K - 1, 0 : T * H].rearrange(
                            "p c k j -> p c j k"
                        ),
                        axis=mybir.AxisListType.X,
                        op=AL.add,
                    )
                    # lus = mu_u * vec - luacc(scaled)
                    nc.vector.scalar_tensor_tensor(
                        out=lus[:, b],
                        in0=vec[:, b],
                        scalar=float(mu_u_val),
                        in1=luacc[:].rearrange("p c t h -> p t h c"),
                        op0=AL.mult,
                        op1=AL.subtract,
                    )
                    # Ldr: k=0..9, t=0..10 -> out t=1..11
                    for c in range(C):
                        nc.vector.tensor_tensor(
                            out=prod[:, c, 0:K, 0 : (T - 1) * H].rearrange(
                                "p k (t h) -> p k t h", t=T - 1
                            ),
                            in0=gf[:, :, 0 : T - 1, :, c],
                            in1=dwt[:],
                            op=AL.mult,
                        )
                    dracc = accp.tile([CH, C, T - 1, H], f32, tag="dracc")
                    nc.vector.tensor_reduce(
                        out=dracc[:],
                        in_=prod[:, :, 0:K, 0 : (T - 1) * H].rearrange(
                            "p c k j -> p c j k"
                        ),
                        axis=mybir.AxisListType.X,
                        op=AL.add,
                    )
                    ldc = smallp.tile([CH, T, H, C], f32, tag="ldc")
                    nc.vector.memset(ldc[:, 0], 0.0)
                    nc.vector.tensor_tensor(
                        out=ldc[:, 1:],
                        in0=vec[:, b, 1:],
                        in1=dracc[:].rearrange("p c t h -> p t h c"),
                        op=AL.subtract,
                    )
                    nc.sync.dma_start(
                        out=ag_in[b * CH : (b + 1) * CH, :],
                        in_=ldc[:].rearrange("p t h c -> p (t h c)"),
                    )
                    if mode == "init":
                        nc.vector.tensor_copy(out=phi[:, b], in_=ldc[:])
                    elif mode == "boundary":
                        rho_v, thr = bctx
                        st = smallp.tile([CH, T, H, C], f32, tag="st")
                        nc.vector.scalar_tensor_tensor(
                            out=st[:], in0=gam[:, b], scalar=float(-1.0 / rho_v),
                            in1=ldc[:], op0=AL.mult, op1=AL.add,
                        )
                        t1 = smallp.tile([CH, T, H, C], f32, tag="t1")
                        nc.vector.tensor_scalar(
                            out=t1[:], in0=st[:], scalar1=float(-thr),
                            scalar2=0.0, op0=AL.add, op1=AL.max,
                        )
                        t2 = smallp.tile([CH, T, H, C], f32, tag="t2")
                        nc.vector.tensor_scalar(
                            out=t2[:], in0=st[:], scalar1=float(thr),
                            scalar2=0.0, op0=AL.add, op1=AL.min,
                        )
                        nc.vector.tensor_tensor(
                            out=phi[:, b], in0=t1[:], in1=t2[:], op=AL.add
                        )
                        # gamma += rho*(phi - ldr)
                        g1 = smallp.tile([CH, T, H, C], f32, tag="g1")
                        nc.vector.tensor_tensor(
                            out=g1[:], in0=phi[:, b], in1=ldc[:], op=AL.subtract
                        )
                        nc.vector.scalar_tensor_tensor(
                            out=gam[:, b], in0=g1[:], scalar=float(rho_v),
                            in1=gam[:, b], op0=AL.mult, op1=AL.add,
                        )
                nc.gpsimd.collective_compute(
                    "AllGather", AL.bypass, replica_groups=rg,
                    ins=[ag_in[:, :].opt()], outs=[ltab[:, :].opt()],
                )
                return ltab

            # ---------------- reverse pass ----------------
            def rev_pass(tab, combine):
                """combine(b, scat, wch_getter) writes results for chunk b.
                scat: [CH, C, T, H] tile; scat[:, :, T-1] is zero;
                scat[:, :, t] = sum_d RW*g[t+1] for t=0..10."""
                for b in range(NCHUNK):
                    db = int(D[b])
                    scat = scatp.tile([CH, C, T, H], f32, tag="scat")
                    nc.vector.memset(scat[:, :, T - 1], 0.0)
                    nrounds = max(1, (db + RD - 1) // RD)
                    for r in range(nrounds):
                        lo = r * RD
                        cnt = min(RD, db - lo)
                        if cnt <= 0:
                            nc.vector.memset(scat[:, :, 0 : T - 1], 0.0)
                            break
                        rwt = wtp.tile([CH, RD, T - 1, H], f32, tag="rwt")
                        nc.sync.dma_start(
                            out=rwt[:, 0:cnt].rearrange("p d t h -> p (d t h)"),
                            in_=rw[
                                b * CH : (b + 1) * CH,
                                (doff[b] + lo) * (T - 1) * H : (doff[b] + lo + cnt) * (T - 1) * H,
                            ],
                        )
                        gr = grp.tile([CH, RD, T, H, C], f32, tag="gr")
                        nc.gpsimd.dma_gather(
                            out_ap=gr[:, 0:cnt].rearrange("p d t h c -> p d (t h c)"),
                            in_ap=tab[:, :],
                            idxs_ap=ridxr[:, (doff[b] + lo) * 8 : (doff[b] + lo + cnt) * 8],
                            num_idxs=CH * cnt,
                            num_idxs_reg=CH * cnt,
                            elem_size=TW,
                            single_packet=False,
                        )
                        prod = prodp.tile([CH, C, RD, 48], f32, tag="prod")
                        for c in range(C):
                            nc.vector.tensor_tensor(
                                out=prod[:, c, 0:cnt, 0 : (T - 1) * H].rearrange(
                                    "p d (t h) -> p d t h", t=T - 1
                                ),
                                in0=gr[:, 0:cnt, 1:, :, c],
                                in1=rwt[:, 0:cnt],
                                op=AL.mult,
                            )
                        if r == 0:
                            nc.vector.tensor_reduce(
                                out=scat[:, :, 0 : T - 1],
                                in_=prod[:, :, 0:cnt, 0 : (T - 1) * H].rearrange(
                                    "p c d j -> p c j d"
                                ),
                                axis=mybir.AxisListType.X,
                                op=AL.add,
                            )
                        else:
                            sc2 = accp.tile([CH, C, T - 1, H], f32, tag="sc2")
                            nc.vector.tensor_reduce(
                                out=sc2[:],
                                in_=prod[:, :, 0:cnt, 0 : (T - 1) * H].rearrange(
                                    "p c d j -> p c j d"
                                ),
                                axis=mybir.AxisListType.X,
                                op=AL.add,
                            )
                            nc.vector.tensor_tensor(
                                out=scat[:, :, 0 : T - 1],
                                in0=scat[:, :, 0 : T - 1],
                                in1=sc2[:],
                                op=AL.add,
                            )
                    combine(b, scat)

            def self_gather(tab, b):
                wch = smallp.tile([CH, 1, T, H, C], f32, tag="wchg")
                nc.gpsimd.dma_gather(
                    out_ap=wch[:].rearrange("p o t h c -> p o (t h c)"),
                    in_ap=tab[:, :],
                    idxs_ap=sidxr[:, b * 8 : (b + 1) * 8],
                    num_idxs=CH,
                    num_idxs_reg=CH,
                    elem_size=TW,
                    single_packet=False,
                )
                return wch[:, 0]

            def cldr_from(scat, wch, qt):
                """qt[:] = Ldr_T: qt[t=0] = -scat[0]; qt[1:] = w[1:] - scat[1:12]
                (scat[T-1] is zero so t=11 passes w through)."""
                nc.vector.tensor_tensor(
                    out=qt[:, 1:],
                    in0=wch[:, 1:],
                    in1=scat[:, :, 1:].rearrange("p c t h -> p t h c"),
                    op=AL.subtract,
                )
                nc.vector.tensor_scalar_mul(
                    qt[:, 0],
                    scat[:, :, 0].rearrange("p c h -> p h c"),
                    -1.0,
                )

            # ================= program =================
            ldr_tab = None  # iteration-0 rhs/r-init is done on host

            flat = lambda ap: ap.rearrange("p t h c -> p (t h c)")

            for i in range(ADMM):
                rho_v = sc["rho"][i]
                cc = sc["mu_d2"][i] + rho_v / 2.0
                if i == 0:
                    nc.vector.tensor_copy(
                        out=ps[:].rearrange("p ch t h c -> p (ch t h c)"),
                        in_=rs[:].rearrange("p ch t h c -> p (ch t h c)"),
                    )
                # ---- rhs: r = LdrT(rho*phi + gamma)/2 + Hty ----
                if i == 0:
                    pass
                else:
                    ag_in, wtab = new_ag()
                    for b in range(NCHUNK):
                        wch = smallp.tile([CH, T, H, C], f32, tag="wch")
                        nc.vector.scalar_tensor_tensor(
                            out=wch[:], in0=phi[:, b], scalar=float(rho_v),
                            in1=gam[:, b], op0=AL.mult, op1=AL.add,
                        )
                        nc.sync.dma_start(
                            out=ag_in[b * CH : (b + 1) * CH, :], in_=flat(wch[:])
                        )
                    nc.gpsimd.collective_compute(
                        "AllGather", AL.bypass, replica_groups=rg,
                        ins=[ag_in[:, :].opt()], outs=[wtab[:, :].opt()],
                    )

                def rhs_combine(b, scat, _i=i, _rho=rho_v):
                    wch = smallp.tile([CH, T, H, C], f32, tag="wch")
                    nc.vector.scalar_tensor_tensor(
                        out=wch[:], in0=phi[:, b], scalar=float(_rho),
                        in1=gam[:, b], op0=AL.mult, op1=AL.add,
                    )
                    cldr_from(scat, wch, rs[:, b])
                    nc.vector.scalar_tensor_tensor(
                        out=rs[:, b, 0:mask].rearrange("p t h c -> p (t h c)"),
                        in0=rs[:, b, 0:mask].rearrange("p t h c -> p (t h c)"),
                        scalar=0.5,
                        in1=hty[:, b].rearrange("p t h c -> p (t h c)"),
                        op0=AL.mult,
                        op1=AL.add,
                    )
                    nc.vector.tensor_scalar_mul(
                        rs[:, b, mask:].rearrange("p t h c -> p (t h c)"),
                        rs[:, b, mask:].rearrange("p t h c -> p (t h c)"),
                        0.5,
                    )

                if i > 0:
                    rev_pass(wtab, rhs_combine)

                # ---- r -= lhs(x) ----
                def rinit_combine(b, scat, _cc=cc, _lt=ldr_tab):
                    wch = self_gather(_lt, b)
                    qt = smallp.tile([CH, T, H, C], f32, tag="qt")
                    cldr_from(scat, wch, qt)
                    nc.vector.scalar_tensor_tensor(
                        out=flat(rs[:, b]), in0=flat(qt[:]), scalar=float(-_cc),
                        in1=flat(rs[:, b]), op0=AL.mult, op1=AL.add,
                    )
                    nc.vector.tensor_tensor(
                        out=flat(rs[:, b]), in0=flat(rs[:, b]),
                        in1=flat(lus[:, b]), op=AL.subtract,
                    )
                    nc.vector.tensor_tensor(
                        out=rs[:, b, 0:mask],
                        in0=rs[:, b, 0:mask],
                        in1=xs[:, b, 0:mask],
                        op=AL.subtract,
                    )

                if i > 0:
                    rev_pass(ldr_tab, rinit_combine)
                    nc.vector.tensor_copy(
                        out=ps[:].rearrange("p ch t h c -> p (ch t h c)"),
                        in_=rs[:].rearrange("p ch t h c -> p (ch t h c)"),
                    )

                # ---- CG ----
                for j in range(CG):
                    al = sc["alpha"][i][j]  # list of H floats
                    be = sc["beta"][i][j]
                    al_eq = all(a == al[0] for a in al)
                    be_eq = all(b2 == be[0] for b2 in be)
                    if j == CG - 1:
                        # only x += a*p
                        if al_eq:
                            nc.vector.scalar_tensor_tensor(
                                out=xs[:].rearrange("p ch t h c -> p (ch t h c)"),
                                in0=ps[:].rearrange("p ch t h c -> p (ch t h c)"),
                                scalar=float(al[0]),
                                in1=xs[:].rearrange("p ch t h c -> p (ch t h c)"),
                                op0=AL.mult, op1=AL.add,
                            )
                        else:
                            for h in range(H):
                                nc.vector.scalar_tensor_tensor(
                                    out=xs[:, :, :, h],
                                    in0=ps[:, :, :, h],
                                    scalar=float(al[h]),
                                    in1=xs[:, :, :, h],
                                    op0=AL.mult, op1=AL.add,
                                )
                        break
                    ptab = run_ag(ps[:].rearrange("p ch t h c -> p ch (t h c)"))
                    ltab_p = fwd_pass(ptab, sc["mu_u"][i], ps, "cg")

                    def cg_combine(b, scat, _cc=cc, _al=al, _be=be, _j=j,
                                   _lt=ltab_p, _aleq=al_eq, _beeq=be_eq):
                        wch = self_gather(_lt, b)
                        qt = smallp.tile([CH, T, H, C], f32, tag="qt")
                        cldr_from(scat, wch, qt)
                        # q = cc*cldr + lus
                        nc.vector.scalar_tensor_tensor(
                            out=flat(qt[:]), in0=flat(qt[:]), scalar=float(_cc),
                            in1=flat(lus[:, b]), op0=AL.mult, op1=AL.add,
                        )
                        # r -= a*q ; r[t<mask] -= a*p
                        if _aleq:
                            nc.vector.scalar_tensor_tensor(
                                out=flat(rs[:, b]), in0=flat(qt[:]),
                                scalar=float(-_al[0]), in1=flat(rs[:, b]),
                                op0=AL.mult, op1=AL.add,
                            )
                            nc.vector.scalar_tensor_tensor(
                                out=rs[:, b, 0:mask].rearrange("p t h c -> p (t h c)"),
                                in0=ps[:, b, 0:mask].rearrange("p t h c -> p (t h c)"),
                                scalar=float(-_al[0]),
                                in1=rs[:, b, 0:mask].rearrange("p t h c -> p (t h c)"),
                                op0=AL.mult, op1=AL.add,
                            )
                            nc.vector.scalar_tensor_tensor(
                                out=flat(xs[:, b]), in0=flat(ps[:, b]),
                                scalar=float(_al[0]), in1=flat(xs[:, b]),
                                op0=AL.mult, op1=AL.add,
                            )
                        else:
                            for h in range(H):
                                fl = lambda ap: ap
                                nc.vector.scalar_tensor_tensor(
                                    out=fl(rs[:, b, :, h]), in0=fl(qt[:, :, h]),
                                    scalar=float(-_al[h]), in1=fl(rs[:, b, :, h]),
                                    op0=AL.mult, op1=AL.add,
                                )
                                nc.vector.scalar_tensor_tensor(
                                    out=fl(rs[:, b, 0:mask, h]),
                                    in0=fl(ps[:, b, 0:mask, h]),
                                    scalar=float(-_al[h]),
                                    in1=fl(rs[:, b, 0:mask, h]),
                                    op0=AL.mult, op1=AL.add,
                                )
                                nc.vector.scalar_tensor_tensor(
                                    out=fl(xs[:, b, :, h]), in0=fl(ps[:, b, :, h]),
                                    scalar=float(_al[h]), in1=fl(xs[:, b, :, h]),
                                    op0=AL.mult, op1=AL.add,
                                )
                        # p = r + b*p
                        if _beeq:
                            nc.vector.scalar_tensor_tensor(
                                out=flat(ps[:, b]), in0=flat(ps[:, b]),
                                scalar=float(_be[0]), in1=flat(rs[:, b]),
                                op0=AL.mult, op1=AL.add,
                            )
                        else:
                            for h in range(H):
                                fl = lambda ap: ap
                                nc.vector.scalar_tensor_tensor(
                                    out=fl(ps[:, b, :, h]), in0=fl(ps[:, b, :, h]),
                                    scalar=float(_be[h]), in1=fl(rs[:, b, :, h]),
                                    op0=AL.mult, op1=AL.add,
                                )

                    rev_pass(ltab_p, cg_combine)

                # ---- boundary ----
                if i < ADMM - 1:
                    xtab = run_ag(xs[:].rearrange("p ch t h c -> p ch (t h c)"))
                    thr = sc["mu_d1"][i] / sc["rho"][i]
                    ldr_tab = fwd_pass(
                        xtab, sc["mu_u"][i + 1], xs, "boundary",
                        bctx=(sc["rho"][i], thr),
                    )

            # ---- output: out[n, t*C+c] = sum_h comb[h]*x ----
            for b in range(NCHUNK):
                oc = smallp.tile([CH, T, C], f32, tag="oc")
                nc.vector.tensor_scalar(
                    out=oc[:],
                    in0=xs[:, b, :, 0],
                    scalar1=float(sc["comb"][0]),
                    scalar2=None,
                    op0=AL.mult,
                )
                for h in range(1, H):
                    nc.vector.scalar_tensor_tensor(
                        out=oc[:],
                        in0=xs[:, b, :, h],
                        scalar=float(sc["comb"][h]),
                        in1=oc[:],
                        op0=AL.mult, op1=AL.add,
                    )
                nc.sync.dma_start(
                    out=outp[b * CH : (b + 1) * CH, :],
                    in_=oc[:].rearrange("p t c -> p (t c)"),
                )

    assert _ag_z[0] == n_ag, (_ag_z[0], n_ag)
    nc.compile()
    return nc


# ---------------- full kernel entry ----------------
def _wrap16(vals):
    """Wrap a flat idx list: pos j -> (partition j%16, col j//16); tiled to
    128 partitions (each GpSimd core reads its own 16-partition copy)."""
    v = np.ascontiguousarray(vals.astype(np.int16))
    w = v.reshape(-1, 16).T
    return np.tile(w, (8, 1))


def _make_in_maps(inputs, prep, r0, phi0):
    x0 = prep["x0"]
    D = prep["D"]
    doff = np.concatenate([[0], np.cumsum(D)]).astype(int)
    in_maps = []
    for c in range(NCORE):
        sl = slice(c * S, (c + 1) * S)
        fwd = prep["fwd_idx"][sl]  # [S, K]
        # fwd wrapped: per chunk, position i = k*128+p
        fw = np.concatenate(
            [fwd[b * CH : (b + 1) * CH].T.reshape(-1) for b in range(NCHUNK)]
        )
        # rev wrapped: per chunk b / round r, position i = d*128+p
        rev = prep["rev_idx"][sl]  # [S, SD]
        rparts = []
        for b in range(NCHUNK):
            sub = rev[b * CH : (b + 1) * CH, doff[b] : doff[b + 1]]  # [128, D[b]]
            rparts.append(sub.T.reshape(-1))
        rv = np.concatenate(rparts) if rparts else np.zeros(0, np.int64)
        sv = c * S + np.arange(S)
        in_maps.append(
            {
                "x0s": x0[sl],
                "r0s": r0[sl],
                "phi0s": phi0[sl],
                "fwdw": _wrap16(fw),
                "selfw": _wrap16(sv),
                "revw": _wrap16(rv),
                "uw": prep["UW"][sl],
                "dw": prep["DW"][sl],
                "rw": prep["RW"][sl],
            }
        )
    return in_maps


def _scalars(inputs):
    return dict(
        mu_u=[float(v) for v in np.asarray(inputs["mu_u"])],
        mu_d1=[float(v) for v in np.asarray(inputs["mu_d1"])],
        mu_d2=[float(v) for v in np.asarray(inputs["mu_d2"])],
        rho=[float(v) for v in np.asarray(inputs["rho"])],
        alpha=[
            [[float(x) for x in np.asarray(inputs["alpha_x"])[i, j, :, 0]]
             for j in range(CG)]
            for i in range(ADMM)
        ],
        beta=[
            [[float(x) for x in np.asarray(inputs["beta_x"])[i, j, :, 0]]
             for j in range(CG)]
            for i in range(ADMM)
        ],
        comb=[float(v) for v in np.asarray(inputs["comb_weights"])],
        mask=int(inputs["mask"]),
    )


def _assemble_out(results, prep):
    out_new = np.concatenate(
        [np.asarray(results[c]["out"]) for c in range(NCORE)], axis=0
    )  # (NP_, T*C)
    out = np.zeros((1, T, N, C), np.float32)
    real = prep["real"]
    out[0, :, prep["orig"][real], :] = out_new[real].reshape(-1, T, C)
    return out


def _maybe_bake_mu_u(prep, sc):
    """If mu_u is constant across iterations, pre-scale UW on the host and
    drop the per-chunk device scale op."""
    if all(v == sc["mu_u"][0] for v in sc["mu_u"]) and sc["mu_u"][0] != 1.0:
        prep["UW"] = (prep["UW"] * np.float32(sc["mu_u"][0])).astype(np.float32)
        sc["mu_u_baked"] = True


def kernel(**inputs) -> np.ndarray:
    from concourse.bass_utils import run_bass_kernel_spmd

    prep = build_prep(
        inputs["y"], inputs["u_ew"], inputs["d_ew"], inputs["nearest_nodes"]
    )
    sc = _scalars(inputs)
    r0, phi0 = host_init(prep, sc)
    _maybe_bake_mu_u(prep, sc)
    nc = build_bass(sc, prep["D"], prep["SD"])
    in_maps = _make_in_maps(inputs, prep, r0, phi0)
    res = run_bass_kernel_spmd(nc, in_maps, core_ids=list(range(NCORE)))
    return _assemble_out(res.results, prep)
